# revision 7
# baseline (speedup 1.0000x reference)
"""Trainium2 Bass kernel for ChunkedFrameEncoder (segment_reduce).

Strategy: frame-sharded data parallelism over 8 NeuronCores.
  - Core c owns frames [c*2048, (c+1)*2048).
  - Host-side sharding prep: sort chunk indices by frame_id, group chunks
    into per-(128-frame)-block buckets padded to a multiple of 128 so the
    device program is fully static and identical across cores (SPMD).
  - On device per chunk-tile [128, 512]: strided DVE reduce (mean over
    chunk_len), PE transpose, fp32r matmul with [W_enc/64; b_enc], GELU,
    then a weighted one-hot scatter-matmul into a PSUM accumulator
    holding [sum_emb | sum_w] for the current frame block.
  - Per frame block: pooled = sum_emb*r + ticker_emb[tid]*(sum_w*r),
    concat log1p(frame_scalars) + ones column, transpose + 3 fp32r
    matmuls against [W_proj; b_proj], LayerNorm + GELU, DMA out.
  - No collectives needed (each core fully owns its frames).
"""

import os
import sys
from contextlib import ExitStack

import numpy as np

sys.path.insert(0, "/opt/trn_rl_repo")

import concourse.bass as bass  # noqa: E402
import concourse.bacc as bacc  # noqa: E402
import concourse.mybir as mybir  # noqa: E402
import concourse.tile as tile  # noqa: E402
from concourse.masks import make_identity  # noqa: E402

P = 128
N_CORES = 8
ENC = 256
DM = 512
NSC = 3
F_DIM = 8
CHUNK_LEN = 64
AF = mybir.ActivationFunctionType
ALU = mybir.AluOpType
DT = mybir.dt


def build_nc(TPB, FB=16, n_tickers=4096, ln_trivial=True, reps=1,
             bf16=False):
    """Build the per-core Bass program.

    TPB: chunk tiles per frame block. FB: frame blocks per core.
    bf16: chunks/mean/encoder/scatter in bf16 (halves chunk DMA).
    """
    T = FB * TPB
    KPAD = T * P
    f32, f32r, i32 = DT.float32, DT.float32r, DT.int32
    bf = DT.bfloat16
    chdt = bf if bf16 else f32

    nc = bacc.Bacc("TRN2", target_bir_lowering=False, debug=False,
                   num_devices=N_CORES)

    ch_d = nc.dram_tensor("ch", [KPAD, CHUNK_LEN * F_DIM], chdt,
                          kind="ExternalInput").ap()
    fid_d = nc.dram_tensor("fid", [P, T], f32, kind="ExternalInput").ap()
    w_d = nc.dram_tensor("w", [P, T], f32, kind="ExternalInput").ap()
    tid_d = nc.dram_tensor("tid", [P, FB], i32, kind="ExternalInput").ap()
    fs_d = nc.dram_tensor("fs", [P, FB * NSC], f32, kind="ExternalInput").ap()
    te_d = nc.dram_tensor("te", [n_tickers, ENC], f32,
                          kind="ExternalInput").ap()
    if bf16:
        wext_d = nc.dram_tensor("wext", [F_DIM + 1, ENC + 2], bf,
                                kind="ExternalInput").ap()
    else:
        wext_d = nc.dram_tensor("wext", [F_DIM + 1, ENC + 2], f32r,
                                kind="ExternalInput").ap()
    wp0_d = nc.dram_tensor("wp0", [P, DM], f32r, kind="ExternalInput").ap()
    wp1_d = nc.dram_tensor("wp1", [P, DM], f32r, kind="ExternalInput").ap()
    wp2_d = nc.dram_tensor("wp2", [4, DM], f32r, kind="ExternalInput").ap()
    if not ln_trivial:
        g_d = nc.dram_tensor("grep", [P, DM], f32, kind="ExternalInput").ap()
        b_d = nc.dram_tensor("brep", [P, DM], f32, kind="ExternalInput").ap()
    y_d = nc.dram_tensor("y", [FB * P, DM], f32, kind="ExternalOutput").ap()

    with tile.TileContext(nc) as tc, ExitStack() as ctx:
        const = ctx.enter_context(tc.tile_pool(name="const", bufs=1))

        ident = const.tile([P, P], f32)
        make_identity(nc, ident[:])
        ident_b = const.tile([P, P], DT.bfloat16)
        make_identity(nc, ident_b[:])
        iota_i = const.tile([P, P], i32)
        nc.gpsimd.iota(iota_i[:], pattern=[[1, P]], base=0,
                       channel_multiplier=0)
        iota_f = const.tile([P, P], f32)
        nc.vector.tensor_copy(iota_f[:], iota_i[:])

        if bf16:
            wext = const.tile([F_DIM + 1, ENC + 2], bf)
        else:
            wext = const.tile([F_DIM + 1, ENC + 2], f32r)
        nc.sync.dma_start(wext[:], wext_d[:])
        wp0 = const.tile([P, DM], f32r)
        nc.sync.dma_start(wp0[:], wp0_d[:])
        wp1 = const.tile([P, DM], f32r)
        nc.sync.dma_start(wp1[:], wp1_d[:])
        wp2 = const.tile([4, DM], f32r)
        nc.sync.dma_start(wp2[:], wp2_d[:])
        fid_sb = const.tile([P, T], f32)
        nc.sync.dma_start(fid_sb[:], fid_d[:])
        w_sb = const.tile([P, T], f32)
        nc.sync.dma_start(w_sb[:], w_d[:])
        fs_sb = const.tile([P, FB * NSC], f32)
        nc.sync.dma_start(fs_sb[:], fs_d[:])
        tid_sb = const.tile([P, FB], i32)
        nc.sync.dma_start(tid_sb[:], tid_d[:])
        if not ln_trivial:
            grep = const.tile([P, DM], f32)
            nc.sync.dma_start(grep[:], g_d[:])
            brep = const.tile([P, DM], f32)
            nc.sync.dma_start(brep[:], b_d[:])

        te_all = const.tile([P, FB * ENC], f32)
        for b in range(FB):
            nc.gpsimd.indirect_dma_start(
                out=te_all[:, b * ENC:(b + 1) * ENC],
                out_offset=None,
                in_=te_d[:],
                in_offset=bass.IndirectOffsetOnAxis(ap=tid_sb[:, b:b + 1],
                                                    axis=0),
            )

        ch_pool = ctx.enter_context(tc.tile_pool(name="chp", bufs=3))
        small = ctx.enter_context(tc.tile_pool(name="small", bufs=3))
        embp = ctx.enter_context(tc.tile_pool(name="embp", bufs=3))
        ohp = ctx.enter_context(tc.tile_pool(name="ohp", bufs=3))
        combp = ctx.enter_context(tc.tile_pool(name="combp", bufs=2))
        outp = ctx.enter_context(tc.tile_pool(name="outp", bufs=2))
        tinyp = ctx.enter_context(tc.tile_pool(name="tinyp", bufs=4))
        psA = ctx.enter_context(tc.tile_pool(name="psA", bufs=2, space="PSUM"))
        psMT = ctx.enter_context(tc.tile_pool(name="psMT", bufs=1,
                                              space="PSUM"))
        psACC = ctx.enter_context(tc.tile_pool(name="psACC", bufs=2,
                                               space="PSUM"))
        psCT = ctx.enter_context(tc.tile_pool(name="psCT", bufs=2,
                                              space="PSUM"))
        psY = ctx.enter_context(tc.tile_pool(name="psY", bufs=1,
                                             space="PSUM"))

        for _rep in range(reps):
            for b in range(FB):
                acc = psACC.tile([P, ENC + 2], f32, tag="acc")
                for j in range(TPB):
                    i = b * TPB + j
                    cht = ch_pool.tile([P, CHUNK_LEN * F_DIM], chdt,
                                       tag="cht")
                    nc.sync.dma_start(cht[:], ch_d[i * P:(i + 1) * P, :])

                    if bf16:
                        # t-fold then strided reduce (sum; /64 in W_ext)
                        fold = small.tile([P, 256], bf, tag="fold")
                        nc.vector.tensor_tensor(
                            out=fold[:], in0=cht[:, 0:256],
                            in1=cht[:, 256:512], op=ALU.add)
                        me = small.tile([P, F_DIM + 1], bf, tag="me")
                        with nc.allow_low_precision("bf16 mean; fp32 internal"):
                            nc.vector.tensor_reduce(
                                out=me[:, 0:F_DIM],
                                in_=fold[:].rearrange("p (t c) -> p c t",
                                                      c=F_DIM),
                                axis=mybir.AxisListType.X, op=ALU.add)
                        nc.gpsimd.memset(me[:, F_DIM:F_DIM + 1], 1.0)
                        mt_ps = psMT.tile([F_DIM + 1, P], bf, tag="mt")
                        nc.tensor.transpose(mt_ps[:], me[:], ident_b[:])
                        mt = small.tile([F_DIM + 1, P], bf, tag="mt_sb")
                        nc.scalar.copy(mt[:], mt_ps[:])
                        h_ps = psA.tile([P, ENC + 2], f32, tag="h")
                        nc.tensor.matmul(h_ps[:], lhsT=mt[:], rhs=wext[:],
                                         start=True, stop=True)
                        emb = embp.tile([P, ENC + 2], bf, tag="emb")
                        oh = ohp.tile([P, P], bf, tag="oh")
                    else:
                        # mean over chunk_len (sum; /64 is folded into W_ext)
                        me = small.tile([P, F_DIM + 1], f32, tag="me")
                        nc.vector.tensor_reduce(
                            out=me[:, 0:F_DIM],
                            in_=cht[:].rearrange("p (t c) -> p c t", c=F_DIM),
                            axis=mybir.AxisListType.X,
                            op=ALU.add,
                        )
                        nc.gpsimd.memset(me[:, F_DIM:F_DIM + 1], 1.0)

                        mt_ps = psMT.tile([F_DIM + 1, P], f32, tag="mt")
                        nc.tensor.transpose(mt_ps[:], me[:], ident[:])
                        mt = small.tile([F_DIM + 1, P], f32r, tag="mt_sb")
                        nc.scalar.copy(mt[:], mt_ps[:])

                        h_ps = psA.tile([P, ENC + 2], f32, tag="h")
                        nc.tensor.matmul(h_ps[:], lhsT=mt[:],
                                         rhs=wext[:],
                                         start=True, stop=True)
                        emb = embp.tile([P, ENC + 2], f32r, tag="emb")
                        oh = ohp.tile([P, P], f32r, tag="oh")

                    nc.scalar.activation(emb[:, 0:ENC], h_ps[:, 0:ENC],
                                         AF.Gelu)
                    nc.scalar.copy(emb[:, ENC:ENC + 2], h_ps[:, ENC:ENC + 2])

                    nc.gpsimd.tensor_scalar(
                        out=oh[:], in0=iota_f[:],
                        scalar1=fid_sb[:, i:i + 1], scalar2=w_sb[:, i:i + 1],
                        op0=ALU.is_equal, op1=ALU.mult,
                    )

                    nc.tensor.matmul(acc[:], lhsT=oh[:], rhs=emb[:],
                                     start=(j == 0), stop=(j == TPB - 1))

                # ---- frame-block epilogue ----
                swp = tinyp.tile([P, 1], f32, tag="swp")
                nc.vector.tensor_scalar_add(swp[:], acc[:, ENC:ENC + 1], 1e-8)
                rcp = tinyp.tile([P, 1], f32, tag="rcp")
                nc.vector.reciprocal(rcp[:], swp[:])
                alpha = tinyp.tile([P, 1], f32, tag="alpha")
                nc.vector.tensor_scalar(out=alpha[:], in0=acc[:, ENC:ENC + 1],
                                        scalar1=rcp[:], scalar2=None,
                                        op0=ALU.mult)

                comb = combp.tile([P, ENC + NSC + 1], f32, tag="comb")
                nc.scalar.activation(comb[:, 0:ENC], acc[:, 0:ENC], AF.Copy,
                                     scale=rcp[:])
                tes = embp.tile([P, ENC], f32, tag="tes")
                nc.vector.tensor_scalar(out=tes[:],
                                        in0=te_all[:, b * ENC:(b + 1) * ENC],
                                        scalar1=alpha[:], scalar2=None,
                                        op0=ALU.mult)
                nc.vector.tensor_tensor(out=comb[:, 0:ENC],
                                        in0=comb[:, 0:ENC], in1=tes[:],
                                        op=ALU.add)
                nc.scalar.activation(comb[:, ENC:ENC + NSC],
                                     fs_sb[:, b * NSC:(b + 1) * NSC],
                                     AF.Ln, bias=1.0)
                nc.gpsimd.memset(comb[:, ENC + NSC:ENC + NSC + 1], 1.0)

                ct0_ps = psCT.tile([P, P], f32, tag="ct")
                nc.tensor.transpose(ct0_ps[:], comb[:, 0:P], ident[:])
                ct0 = embp.tile([P, P], f32r, tag="ct0")
                nc.scalar.copy(ct0[:], ct0_ps[:])
                ct1_ps = psCT.tile([P, P], f32, tag="ct")
                nc.tensor.transpose(ct1_ps[:], comb[:, P:2 * P], ident[:])
                ct1 = embp.tile([P, P], f32r, tag="ct1")
                nc.vector.tensor_copy(ct1[:], ct1_ps[:])
                ct2_ps = psMT.tile([4, P], f32, tag="mt")
                nc.tensor.transpose(ct2_ps[:], comb[:, 2 * P:2 * P + 4],
                                    ident[:])
                ct2 = small.tile([4, P], f32r, tag="ct2")
                nc.scalar.copy(ct2[:], ct2_ps[:])

                y_ps = psY.tile([P, DM], f32, tag="y")
                nc.tensor.matmul(y_ps[:], lhsT=ct0[:],
                                 rhs=wp0[:],
                                 start=True, stop=False)
                nc.tensor.matmul(y_ps[:], lhsT=ct1[:],
                                 rhs=wp1[:],
                                 start=False, stop=False)
                nc.tensor.matmul(y_ps[:], lhsT=ct2[:],
                                 rhs=wp2[:],
                                 start=False, stop=True)

                # LayerNorm + GELU
                ysb = outp.tile([P, DM], f32, tag="ysb")
                rs = tinyp.tile([P, 1], f32, tag="rs")
                nc.scalar.activation(ysb[:], y_ps[:], AF.Copy,
                                     accum_out=rs[:])
                sq = outp.tile([P, DM], f32, tag="sq")
                ss = tinyp.tile([P, 1], f32, tag="ss")
                nc.scalar.activation(sq[:], y_ps[:], AF.Square,
                                     accum_out=ss[:])
                mu = tinyp.tile([P, 1], f32, tag="mu")
                nc.vector.tensor_scalar(out=mu[:], in0=rs[:],
                                        scalar1=1.0 / DM, scalar2=None,
                                        op0=ALU.mult)
                ex2e = tinyp.tile([P, 1], f32, tag="ex2e")
                nc.vector.tensor_scalar(out=ex2e[:], in0=ss[:],
                                        scalar1=1.0 / DM, scalar2=1e-5,
                                        op0=ALU.mult, op1=ALU.add)
                nmu2 = tinyp.tile([P, 1], f32, tag="nmu2")
                nc.vector.tensor_scalar(out=nmu2[:], in0=mu[:], scalar1=mu[:],
                                        scalar2=-1.0, op0=ALU.mult,
                                        op1=ALU.mult)
                veps = tinyp.tile([P, 1], f32, tag="veps")
                nc.vector.tensor_tensor(out=veps[:], in0=ex2e[:], in1=nmu2[:],
                                        op=ALU.add)
                stdv = tinyp.tile([P, 1], f32, tag="stdv")
                nc.scalar.sqrt(stdv[:], veps[:])
                inv = tinyp.tile([P, 1], f32, tag="inv")
                nc.vector.reciprocal(inv[:], stdv[:])
                nmi = tinyp.tile([P, 1], f32, tag="nmi")
                nc.vector.tensor_scalar(out=nmi[:], in0=mu[:], scalar1=inv[:],
                                        scalar2=-1.0, op0=ALU.mult,
                                        op1=ALU.mult)

                yo = outp.tile([P, DM], f32, tag="yo")
                if ln_trivial:
                    nc.scalar.activation(yo[:], ysb[:], AF.Gelu,
                                         bias=nmi[:], scale=inv[:])
                else:
                    t_ = outp.tile([P, DM], f32, tag="tnorm")
                    nc.scalar.activation(t_[:], ysb[:], AF.Identity,
                                         bias=nmi[:], scale=inv[:])
                    z1 = outp.tile([P, DM], f32, tag="z1")
                    nc.vector.tensor_tensor(out=z1[:], in0=t_[:], in1=grep[:],
                                            op=ALU.mult)
                    z2 = outp.tile([P, DM], f32, tag="z2")
                    nc.vector.tensor_tensor(out=z2[:], in0=z1[:], in1=brep[:],
                                            op=ALU.add)
                    nc.scalar.activation(yo[:], z2[:], AF.Gelu)

                nc.sync.dma_start(y_d[b * P:(b + 1) * P, :], yo[:])

    nc.compile()
    return nc


def prep_inputs(chunks, frame_id, weights, frame_scalars, ticker_ids,
                W_enc, b_enc, ticker_emb, W_proj, b_proj, ln_gamma, ln_beta,
                FB=16, min_tpb=1, bf16=False):
    """Host-side sharding prep. Returns (in_maps, TPB, ln_trivial)."""
    total_k = chunks.shape[0]
    num_frames = FB * P * N_CORES
    frames_per_core = FB * P

    order = np.argsort(frame_id, kind="stable")
    fid_sorted = frame_id[order]
    n_gblocks = num_frames // P
    gblock = fid_sorted // P
    counts = np.bincount(gblock, minlength=n_gblocks)
    TPB = max(int(np.ceil(counts.max() / P)), min_tpb)
    CB = TPB * P
    block_starts = np.zeros(n_gblocks + 1, np.int64)
    block_starts[1:] = np.cumsum(counts)

    T = FB * TPB
    KPAD = T * P
    import ml_dtypes
    chdt_np = ml_dtypes.bfloat16 if bf16 else np.float32
    ch_all = np.zeros((N_CORES, KPAD, CHUNK_LEN * F_DIM), chdt_np)
    fid_rel = np.zeros((N_CORES, KPAD), np.float32)
    w_all = np.zeros((N_CORES, KPAD), np.float32)
    chunks2d = chunks.reshape(total_k, CHUNK_LEN * F_DIM)
    for g in range(n_gblocks):
        c, b = g // FB, g % FB
        s, e = int(block_starts[g]), int(block_starts[g + 1])
        n = e - s
        row0 = b * CB
        idx = order[s:e]
        ch_all[c, row0:row0 + n] = chunks2d[idx].astype(chdt_np)
        fid_rel[c, row0:row0 + n] = (frame_id[idx] - g * P).astype(np.float32)
        w_all[c, row0:row0 + n] = weights[idx]

    fid_r = np.ascontiguousarray(
        fid_rel.reshape(N_CORES, T, P).transpose(0, 2, 1))
    w_r = np.ascontiguousarray(w_all.reshape(N_CORES, T, P).transpose(0, 2, 1))
    tid_r = np.ascontiguousarray(
        ticker_ids.reshape(N_CORES, FB, P).transpose(0, 2, 1)).astype(np.int32)
    fs_r = np.ascontiguousarray(
        frame_scalars.reshape(N_CORES, FB, P, NSC).transpose(0, 2, 1, 3)
        .reshape(N_CORES, P, FB * NSC))

    W_ext = np.zeros((F_DIM + 1, ENC + 2), np.float32)
    W_ext[0:F_DIM, 0:ENC] = W_enc / float(CHUNK_LEN)
    W_ext[F_DIM, 0:ENC] = b_enc
    W_ext[F_DIM, ENC] = 1.0
    if bf16:
        W_ext = W_ext.astype(ml_dtypes.bfloat16)
    W_pj = np.concatenate([W_proj, b_proj[None, :]], axis=0)  # [260, 512]
    wp0 = np.ascontiguousarray(W_pj[0:P])
    wp1 = np.ascontiguousarray(W_pj[P:2 * P])
    wp2 = np.ascontiguousarray(W_pj[2 * P:2 * P + 4])

    ln_trivial = bool(np.all(ln_gamma == 1.0) and np.all(ln_beta == 0.0))

    in_maps = []
    for c in range(N_CORES):
        m = {
            "ch": ch_all[c],
            "fid": fid_r[c],
            "w": w_r[c],
            "tid": tid_r[c],
            "fs": fs_r[c],
            "te": ticker_emb,
            "wext": W_ext,
            "wp0": wp0,
            "wp1": wp1,
            "wp2": wp2,
        }
        if not ln_trivial:
            m["grep"] = np.ascontiguousarray(
                np.broadcast_to(ln_gamma[None, :], (P, DM)))
            m["brep"] = np.ascontiguousarray(
                np.broadcast_to(ln_beta[None, :], (P, DM)))
        in_maps.append(m)
    return in_maps, TPB, ln_trivial


_NC_CACHE = {}


def _get_nc(TPB, ln_trivial, reps=1, bf16=False):
    key = (TPB, ln_trivial, reps, bf16)
    if key not in _NC_CACHE:
        _NC_CACHE[key] = build_nc(TPB, ln_trivial=ln_trivial, reps=reps,
                                  bf16=bf16)
    return _NC_CACHE[key]


def kernel(chunks, frame_id, weights, frame_scalars, num_frames, ticker_ids,
           W_enc, b_enc, ticker_emb, W_proj, b_proj, ln_gamma, ln_beta):
    from concourse.bass_utils import run_bass_kernel_spmd

    chunks = np.asarray(chunks, np.float32)
    frame_id = np.asarray(frame_id).astype(np.int64)
    weights = np.asarray(weights, np.float32)
    frame_scalars = np.asarray(frame_scalars, np.float32)
    num_frames = int(num_frames)
    ticker_ids = np.asarray(ticker_ids).astype(np.int64)
    W_enc = np.asarray(W_enc, np.float32)
    b_enc = np.asarray(b_enc, np.float32)
    ticker_emb = np.asarray(ticker_emb, np.float32)
    W_proj = np.asarray(W_proj, np.float32)
    b_proj = np.asarray(b_proj, np.float32)
    ln_gamma = np.asarray(ln_gamma, np.float32)
    ln_beta = np.asarray(ln_beta, np.float32)

    assert num_frames == FB_TOTAL_FRAMES, num_frames
    assert chunks.shape[1:] == (CHUNK_LEN, F_DIM)

    in_maps, TPB, ln_trivial = prep_inputs(
        chunks, frame_id, weights, frame_scalars, ticker_ids,
        W_enc, b_enc, ticker_emb, W_proj, b_proj, ln_gamma, ln_beta,
        bf16=BF16_DEFAULT)

    nc = _get_nc(TPB, ln_trivial, bf16=BF16_DEFAULT)
    res = run_bass_kernel_spmd(nc, in_maps, core_ids=list(range(N_CORES)))
    y = np.concatenate([res.results[c]["y"] for c in range(N_CORES)], axis=0)
    return y.astype(np.float32)


FB_TOTAL_FRAMES = 16384
BF16_DEFAULT = os.environ.get("KERNEL_BF16", "0") == "1"


# revision 8
# speedup vs baseline: 2.8520x; 2.8520x over previous
"""Trainium2 Bass kernel for ChunkedFrameEncoder (segment_reduce).

Strategy: frame-sharded data parallelism over 8 NeuronCores.
  - Core c owns frames [c*2048, (c+1)*2048).
  - Host-side sharding prep: sort chunk indices by frame_id, group chunks
    into per-(128-frame)-block buckets padded to a multiple of 128 so the
    device program is fully static and identical across cores (SPMD).
  - On device per chunk-tile [128, 512]: strided DVE reduce (mean over
    chunk_len), PE transpose, fp32r matmul with [W_enc/64; b_enc], GELU,
    then a weighted one-hot scatter-matmul into a PSUM accumulator
    holding [sum_emb | sum_w] for the current frame block.
  - Per frame block: pooled = sum_emb*r + ticker_emb[tid]*(sum_w*r),
    concat log1p(frame_scalars) + ones column, transpose + 3 fp32r
    matmuls against [W_proj; b_proj], LayerNorm + GELU, DMA out.
  - No collectives needed (each core fully owns its frames).
"""

import os
import sys
from contextlib import ExitStack

import numpy as np

sys.path.insert(0, "/opt/trn_rl_repo")

import concourse.bass as bass  # noqa: E402
import concourse.bacc as bacc  # noqa: E402
import concourse.mybir as mybir  # noqa: E402
import concourse.tile as tile  # noqa: E402
from concourse.masks import make_identity  # noqa: E402

P = 128
N_CORES = 8
ENC = 256
DM = 512
NSC = 3
F_DIM = 8
CHUNK_LEN = 64
AF = mybir.ActivationFunctionType
ALU = mybir.AluOpType
DT = mybir.dt


def build_nc(TPB, FB=16, n_tickers=4096, ln_trivial=True, reps=1,
             bf16=False):
    """Build the per-core Bass program.

    TPB: chunk tiles per frame block. FB: frame blocks per core.
    bf16: chunks/mean/encoder/scatter in bf16 (halves chunk DMA).
    """
    T = FB * TPB
    KPAD = T * P
    f32, f32r, i32 = DT.float32, DT.float32r, DT.int32
    bf = DT.bfloat16
    chdt = bf if bf16 else f32

    nc = bacc.Bacc("TRN2", target_bir_lowering=False, debug=False,
                   num_devices=N_CORES)

    ch_d = nc.dram_tensor("ch", [KPAD, CHUNK_LEN * F_DIM], chdt,
                          kind="ExternalInput").ap()
    fid_d = nc.dram_tensor("fid", [P, T], f32, kind="ExternalInput").ap()
    w_d = nc.dram_tensor("w", [P, T], f32, kind="ExternalInput").ap()
    tid_d = nc.dram_tensor("tid", [P, FB], i32, kind="ExternalInput").ap()
    fs_d = nc.dram_tensor("fs", [P, FB * NSC], f32, kind="ExternalInput").ap()
    te_d = nc.dram_tensor("te", [n_tickers, ENC], f32,
                          kind="ExternalInput").ap()
    if bf16:
        wext_d = nc.dram_tensor("wext", [F_DIM + 1, ENC + 2], bf,
                                kind="ExternalInput").ap()
    else:
        wext_d = nc.dram_tensor("wext", [F_DIM + 1, ENC + 2], f32r,
                                kind="ExternalInput").ap()
    wp0_d = nc.dram_tensor("wp0", [P, DM], f32r, kind="ExternalInput").ap()
    wp1_d = nc.dram_tensor("wp1", [P, DM], f32r, kind="ExternalInput").ap()
    wp2_d = nc.dram_tensor("wp2", [4, DM], f32r, kind="ExternalInput").ap()
    if not ln_trivial:
        g_d = nc.dram_tensor("grep", [P, DM], f32, kind="ExternalInput").ap()
        b_d = nc.dram_tensor("brep", [P, DM], f32, kind="ExternalInput").ap()
    y_d = nc.dram_tensor("y", [FB * P, DM], f32, kind="ExternalOutput").ap()

    with tile.TileContext(nc) as tc, ExitStack() as ctx:
        const = ctx.enter_context(tc.tile_pool(name="const", bufs=1))

        ident = const.tile([P, P], f32)
        make_identity(nc, ident[:])
        ident_b = const.tile([P, P], DT.bfloat16)
        make_identity(nc, ident_b[:])
        iota_i = const.tile([P, P], i32)
        nc.gpsimd.iota(iota_i[:], pattern=[[1, P]], base=0,
                       channel_multiplier=0)
        iota_f = const.tile([P, P], f32)
        nc.vector.tensor_copy(iota_f[:], iota_i[:])

        if bf16:
            wext = const.tile([F_DIM + 1, ENC + 2], bf)
        else:
            wext = const.tile([F_DIM + 1, ENC + 2], f32r)
        nc.sync.dma_start(wext[:], wext_d[:])
        wp0 = const.tile([P, DM], f32r)
        nc.sync.dma_start(wp0[:], wp0_d[:])
        wp1 = const.tile([P, DM], f32r)
        nc.sync.dma_start(wp1[:], wp1_d[:])
        wp2 = const.tile([4, DM], f32r)
        nc.sync.dma_start(wp2[:], wp2_d[:])
        fid_sb = const.tile([P, T], f32)
        nc.sync.dma_start(fid_sb[:], fid_d[:])
        w_sb = const.tile([P, T], f32)
        nc.sync.dma_start(w_sb[:], w_d[:])
        fs_sb = const.tile([P, FB * NSC], f32)
        nc.sync.dma_start(fs_sb[:], fs_d[:])
        tid_sb = const.tile([P, FB], i32)
        nc.sync.dma_start(tid_sb[:], tid_d[:])
        if not ln_trivial:
            grep = const.tile([P, DM], f32)
            nc.sync.dma_start(grep[:], g_d[:])
            brep = const.tile([P, DM], f32)
            nc.sync.dma_start(brep[:], b_d[:])

        te_all = const.tile([P, FB * ENC], f32)
        for b in range(FB):
            nc.gpsimd.indirect_dma_start(
                out=te_all[:, b * ENC:(b + 1) * ENC],
                out_offset=None,
                in_=te_d[:],
                in_offset=bass.IndirectOffsetOnAxis(ap=tid_sb[:, b:b + 1],
                                                    axis=0),
            )

        ch_pool = ctx.enter_context(tc.tile_pool(name="chp", bufs=6))
        small = ctx.enter_context(tc.tile_pool(name="small", bufs=3))
        embp = ctx.enter_context(tc.tile_pool(name="embp", bufs=4))
        ohp = ctx.enter_context(tc.tile_pool(name="ohp", bufs=4))
        combp = ctx.enter_context(tc.tile_pool(name="combp", bufs=2))
        outp = ctx.enter_context(tc.tile_pool(name="outp", bufs=2))
        tinyp = ctx.enter_context(tc.tile_pool(name="tinyp", bufs=4))
        psA = ctx.enter_context(tc.tile_pool(name="psA", bufs=2, space="PSUM"))
        psMT = ctx.enter_context(tc.tile_pool(name="psMT", bufs=1,
                                              space="PSUM"))
        psACC = ctx.enter_context(tc.tile_pool(name="psACC", bufs=2,
                                               space="PSUM"))
        psCT = ctx.enter_context(tc.tile_pool(name="psCT", bufs=2,
                                              space="PSUM"))
        psY = ctx.enter_context(tc.tile_pool(name="psY", bufs=1,
                                             space="PSUM"))

        for _rep in range(reps):
            for b in range(FB):
                acc = psACC.tile([P, ENC + 2], f32, tag="acc")
                for j in range(TPB):
                    i = b * TPB + j
                    cht = ch_pool.tile([P, CHUNK_LEN * F_DIM], chdt,
                                       tag="cht")
                    nc.sync.dma_start(cht[:], ch_d[i * P:(i + 1) * P, :])

                    if bf16:
                        # t-fold then strided reduce (sum; /64 in W_ext)
                        fold = small.tile([P, 256], bf, tag="fold")
                        nc.vector.tensor_tensor(
                            out=fold[:], in0=cht[:, 0:256],
                            in1=cht[:, 256:512], op=ALU.add)
                        me = small.tile([P, F_DIM + 1], bf, tag="me")
                        with nc.allow_low_precision("bf16 mean; fp32 internal"):
                            nc.vector.tensor_reduce(
                                out=me[:, 0:F_DIM],
                                in_=fold[:].rearrange("p (t c) -> p c t",
                                                      c=F_DIM),
                                axis=mybir.AxisListType.X, op=ALU.add)
                        nc.vector.memset(me[:, F_DIM:F_DIM + 1], 1.0)
                        mt_ps = psMT.tile([F_DIM + 1, P], bf, tag="mt")
                        nc.tensor.transpose(mt_ps[:], me[:], ident_b[:])
                        mt = small.tile([F_DIM + 1, P], bf, tag="mt_sb")
                        nc.scalar.copy(mt[:], mt_ps[:])
                        h_ps = psA.tile([P, ENC + 2], f32, tag="h")
                        nc.tensor.matmul(h_ps[:], lhsT=mt[:], rhs=wext[:],
                                         start=True, stop=True)
                        emb = embp.tile([P, ENC + 2], bf, tag="emb")
                        oh = ohp.tile([P, P], bf, tag="oh")
                    else:
                        # mean over chunk_len (sum; /64 is folded into W_ext)
                        me = small.tile([P, F_DIM + 1], f32, tag="me")
                        nc.vector.tensor_reduce(
                            out=me[:, 0:F_DIM],
                            in_=cht[:].rearrange("p (t c) -> p c t", c=F_DIM),
                            axis=mybir.AxisListType.X,
                            op=ALU.add,
                        )
                        nc.gpsimd.memset(me[:, F_DIM:F_DIM + 1], 1.0)

                        mt_ps = psMT.tile([F_DIM + 1, P], f32, tag="mt")
                        nc.tensor.transpose(mt_ps[:], me[:], ident[:])
                        mt = small.tile([F_DIM + 1, P], f32r, tag="mt_sb")
                        nc.scalar.copy(mt[:], mt_ps[:])

                        h_ps = psA.tile([P, ENC + 2], f32, tag="h")
                        nc.tensor.matmul(h_ps[:], lhsT=mt[:],
                                         rhs=wext[:],
                                         start=True, stop=True)
                        emb = embp.tile([P, ENC + 2], f32r, tag="emb")
                        oh = ohp.tile([P, P], f32r, tag="oh")

                    nc.scalar.activation(emb[:, 0:ENC], h_ps[:, 0:ENC],
                                         AF.Gelu)
                    nc.scalar.copy(emb[:, ENC:ENC + 2], h_ps[:, ENC:ENC + 2])

                    nc.vector.tensor_scalar(
                        out=oh[:], in0=iota_f[:],
                        scalar1=fid_sb[:, i:i + 1], scalar2=w_sb[:, i:i + 1],
                        op0=ALU.is_equal, op1=ALU.mult,
                    )

                    nc.tensor.matmul(acc[:], lhsT=oh[:], rhs=emb[:],
                                     start=(j == 0), stop=(j == TPB - 1))

                # ---- frame-block epilogue ----
                swp = tinyp.tile([P, 1], f32, tag="swp")
                nc.vector.tensor_scalar_add(swp[:], acc[:, ENC:ENC + 1], 1e-8)
                rcp = tinyp.tile([P, 1], f32, tag="rcp")
                nc.vector.reciprocal(rcp[:], swp[:])
                alpha = tinyp.tile([P, 1], f32, tag="alpha")
                nc.vector.tensor_scalar(out=alpha[:], in0=acc[:, ENC:ENC + 1],
                                        scalar1=rcp[:], scalar2=None,
                                        op0=ALU.mult)

                comb = combp.tile([P, ENC + NSC + 1], f32, tag="comb")
                nc.scalar.activation(comb[:, 0:ENC], acc[:, 0:ENC], AF.Copy,
                                     scale=rcp[:])
                tes = embp.tile([P, ENC], f32, tag="tes")
                nc.vector.tensor_scalar(out=tes[:],
                                        in0=te_all[:, b * ENC:(b + 1) * ENC],
                                        scalar1=alpha[:], scalar2=None,
                                        op0=ALU.mult)
                nc.vector.tensor_tensor(out=comb[:, 0:ENC],
                                        in0=comb[:, 0:ENC], in1=tes[:],
                                        op=ALU.add)
                nc.scalar.activation(comb[:, ENC:ENC + NSC],
                                     fs_sb[:, b * NSC:(b + 1) * NSC],
                                     AF.Ln, bias=1.0)
                nc.gpsimd.memset(comb[:, ENC + NSC:ENC + NSC + 1], 1.0)

                ct0_ps = psCT.tile([P, P], f32, tag="ct")
                nc.tensor.transpose(ct0_ps[:], comb[:, 0:P], ident[:])
                ct0 = embp.tile([P, P], f32r, tag="ct0")
                nc.scalar.copy(ct0[:], ct0_ps[:])
                ct1_ps = psCT.tile([P, P], f32, tag="ct")
                nc.tensor.transpose(ct1_ps[:], comb[:, P:2 * P], ident[:])
                ct1 = embp.tile([P, P], f32r, tag="ct1")
                nc.vector.tensor_copy(ct1[:], ct1_ps[:])
                ct2_ps = psMT.tile([4, P], f32, tag="mt")
                nc.tensor.transpose(ct2_ps[:], comb[:, 2 * P:2 * P + 4],
                                    ident[:])
                ct2 = small.tile([4, P], f32r, tag="ct2")
                nc.scalar.copy(ct2[:], ct2_ps[:])

                y_ps = psY.tile([P, DM], f32, tag="y")
                nc.tensor.matmul(y_ps[:], lhsT=ct0[:],
                                 rhs=wp0[:],
                                 start=True, stop=False)
                nc.tensor.matmul(y_ps[:], lhsT=ct1[:],
                                 rhs=wp1[:],
                                 start=False, stop=False)
                nc.tensor.matmul(y_ps[:], lhsT=ct2[:],
                                 rhs=wp2[:],
                                 start=False, stop=True)

                # LayerNorm + GELU
                ysb = outp.tile([P, DM], f32, tag="ysb")
                rs = tinyp.tile([P, 1], f32, tag="rs")
                nc.scalar.activation(ysb[:], y_ps[:], AF.Copy,
                                     accum_out=rs[:])
                sq = outp.tile([P, DM], f32, tag="sq")
                ss = tinyp.tile([P, 1], f32, tag="ss")
                nc.scalar.activation(sq[:], y_ps[:], AF.Square,
                                     accum_out=ss[:])
                mu = tinyp.tile([P, 1], f32, tag="mu")
                nc.vector.tensor_scalar(out=mu[:], in0=rs[:],
                                        scalar1=1.0 / DM, scalar2=None,
                                        op0=ALU.mult)
                ex2e = tinyp.tile([P, 1], f32, tag="ex2e")
                nc.vector.tensor_scalar(out=ex2e[:], in0=ss[:],
                                        scalar1=1.0 / DM, scalar2=1e-5,
                                        op0=ALU.mult, op1=ALU.add)
                nmu2 = tinyp.tile([P, 1], f32, tag="nmu2")
                nc.vector.tensor_scalar(out=nmu2[:], in0=mu[:], scalar1=mu[:],
                                        scalar2=-1.0, op0=ALU.mult,
                                        op1=ALU.mult)
                veps = tinyp.tile([P, 1], f32, tag="veps")
                nc.vector.tensor_tensor(out=veps[:], in0=ex2e[:], in1=nmu2[:],
                                        op=ALU.add)
                stdv = tinyp.tile([P, 1], f32, tag="stdv")
                nc.scalar.sqrt(stdv[:], veps[:])
                inv = tinyp.tile([P, 1], f32, tag="inv")
                nc.vector.reciprocal(inv[:], stdv[:])
                nmi = tinyp.tile([P, 1], f32, tag="nmi")
                nc.vector.tensor_scalar(out=nmi[:], in0=mu[:], scalar1=inv[:],
                                        scalar2=-1.0, op0=ALU.mult,
                                        op1=ALU.mult)

                yo = outp.tile([P, DM], f32, tag="yo")
                if ln_trivial:
                    nc.scalar.activation(yo[:], ysb[:], AF.Gelu,
                                         bias=nmi[:], scale=inv[:])
                else:
                    t_ = outp.tile([P, DM], f32, tag="tnorm")
                    nc.scalar.activation(t_[:], ysb[:], AF.Identity,
                                         bias=nmi[:], scale=inv[:])
                    z1 = outp.tile([P, DM], f32, tag="z1")
                    nc.vector.tensor_tensor(out=z1[:], in0=t_[:], in1=grep[:],
                                            op=ALU.mult)
                    z2 = outp.tile([P, DM], f32, tag="z2")
                    nc.vector.tensor_tensor(out=z2[:], in0=z1[:], in1=brep[:],
                                            op=ALU.add)
                    nc.scalar.activation(yo[:], z2[:], AF.Gelu)

                nc.sync.dma_start(y_d[b * P:(b + 1) * P, :], yo[:])

    nc.compile()
    return nc


def prep_inputs(chunks, frame_id, weights, frame_scalars, ticker_ids,
                W_enc, b_enc, ticker_emb, W_proj, b_proj, ln_gamma, ln_beta,
                FB=16, min_tpb=1, bf16=False):
    """Host-side sharding prep. Returns (in_maps, TPB, ln_trivial)."""
    total_k = chunks.shape[0]
    num_frames = FB * P * N_CORES
    frames_per_core = FB * P

    order = np.argsort(frame_id, kind="stable")
    fid_sorted = frame_id[order]
    n_gblocks = num_frames // P
    gblock = fid_sorted // P
    counts = np.bincount(gblock, minlength=n_gblocks)
    TPB = max(int(np.ceil(counts.max() / P)), min_tpb)
    CB = TPB * P
    block_starts = np.zeros(n_gblocks + 1, np.int64)
    block_starts[1:] = np.cumsum(counts)

    T = FB * TPB
    KPAD = T * P
    import ml_dtypes
    chdt_np = ml_dtypes.bfloat16 if bf16 else np.float32
    ch_all = np.zeros((N_CORES, KPAD, CHUNK_LEN * F_DIM), chdt_np)
    fid_rel = np.zeros((N_CORES, KPAD), np.float32)
    w_all = np.zeros((N_CORES, KPAD), np.float32)
    chunks2d = chunks.reshape(total_k, CHUNK_LEN * F_DIM)
    for g in range(n_gblocks):
        c, b = g // FB, g % FB
        s, e = int(block_starts[g]), int(block_starts[g + 1])
        n = e - s
        row0 = b * CB
        idx = order[s:e]
        ch_all[c, row0:row0 + n] = chunks2d[idx].astype(chdt_np)
        fid_rel[c, row0:row0 + n] = (frame_id[idx] - g * P).astype(np.float32)
        w_all[c, row0:row0 + n] = weights[idx]

    fid_r = np.ascontiguousarray(
        fid_rel.reshape(N_CORES, T, P).transpose(0, 2, 1))
    w_r = np.ascontiguousarray(w_all.reshape(N_CORES, T, P).transpose(0, 2, 1))
    tid_r = np.ascontiguousarray(
        ticker_ids.reshape(N_CORES, FB, P).transpose(0, 2, 1)).astype(np.int32)
    fs_r = np.ascontiguousarray(
        frame_scalars.reshape(N_CORES, FB, P, NSC).transpose(0, 2, 1, 3)
        .reshape(N_CORES, P, FB * NSC))

    W_ext = np.zeros((F_DIM + 1, ENC + 2), np.float32)
    W_ext[0:F_DIM, 0:ENC] = W_enc / float(CHUNK_LEN)
    W_ext[F_DIM, 0:ENC] = b_enc
    W_ext[F_DIM, ENC] = 1.0
    if bf16:
        W_ext = W_ext.astype(ml_dtypes.bfloat16)
    W_pj = np.concatenate([W_proj, b_proj[None, :]], axis=0)  # [260, 512]
    wp0 = np.ascontiguousarray(W_pj[0:P])
    wp1 = np.ascontiguousarray(W_pj[P:2 * P])
    wp2 = np.ascontiguousarray(W_pj[2 * P:2 * P + 4])

    ln_trivial = bool(np.all(ln_gamma == 1.0) and np.all(ln_beta == 0.0))

    in_maps = []
    for c in range(N_CORES):
        m = {
            "ch": ch_all[c],
            "fid": fid_r[c],
            "w": w_r[c],
            "tid": tid_r[c],
            "fs": fs_r[c],
            "te": ticker_emb,
            "wext": W_ext,
            "wp0": wp0,
            "wp1": wp1,
            "wp2": wp2,
        }
        if not ln_trivial:
            m["grep"] = np.ascontiguousarray(
                np.broadcast_to(ln_gamma[None, :], (P, DM)))
            m["brep"] = np.ascontiguousarray(
                np.broadcast_to(ln_beta[None, :], (P, DM)))
        in_maps.append(m)
    return in_maps, TPB, ln_trivial


_NC_CACHE = {}


def _get_nc(TPB, ln_trivial, reps=1, bf16=False):
    key = (TPB, ln_trivial, reps, bf16)
    if key not in _NC_CACHE:
        _NC_CACHE[key] = build_nc(TPB, ln_trivial=ln_trivial, reps=reps,
                                  bf16=bf16)
    return _NC_CACHE[key]


def kernel(chunks, frame_id, weights, frame_scalars, num_frames, ticker_ids,
           W_enc, b_enc, ticker_emb, W_proj, b_proj, ln_gamma, ln_beta):
    from concourse.bass_utils import run_bass_kernel_spmd

    chunks = np.asarray(chunks, np.float32)
    frame_id = np.asarray(frame_id).astype(np.int64)
    weights = np.asarray(weights, np.float32)
    frame_scalars = np.asarray(frame_scalars, np.float32)
    num_frames = int(num_frames)
    ticker_ids = np.asarray(ticker_ids).astype(np.int64)
    W_enc = np.asarray(W_enc, np.float32)
    b_enc = np.asarray(b_enc, np.float32)
    ticker_emb = np.asarray(ticker_emb, np.float32)
    W_proj = np.asarray(W_proj, np.float32)
    b_proj = np.asarray(b_proj, np.float32)
    ln_gamma = np.asarray(ln_gamma, np.float32)
    ln_beta = np.asarray(ln_beta, np.float32)

    assert num_frames == FB_TOTAL_FRAMES, num_frames
    assert chunks.shape[1:] == (CHUNK_LEN, F_DIM)

    in_maps, TPB, ln_trivial = prep_inputs(
        chunks, frame_id, weights, frame_scalars, ticker_ids,
        W_enc, b_enc, ticker_emb, W_proj, b_proj, ln_gamma, ln_beta,
        bf16=BF16_DEFAULT)

    nc = _get_nc(TPB, ln_trivial, bf16=BF16_DEFAULT)
    res = run_bass_kernel_spmd(nc, in_maps, core_ids=list(range(N_CORES)))
    y = np.concatenate([res.results[c]["y"] for c in range(N_CORES)], axis=0)
    return y.astype(np.float32)


FB_TOTAL_FRAMES = 16384
BF16_DEFAULT = os.environ.get("KERNEL_BF16", "0") == "1"
